# revision 1
# baseline (speedup 1.0000x reference)
"""Trainium2 Bass kernel for nn_CausalWanSelfAttention_45904610460041.

Strategy (8 NeuronCores, full I/O):
  Launch 1 (pair-split): cores 2p/2p+1 share 390 query rows; core 2p
    computes all of wq + wv[:, :768], core 2p+1 all of wk + wv[:, 768:]
    (bf16 matmuls, fp32 PSUM), so the RMS norm stays core-local while
    per-core weight DMA drops from 14.2MB to ~7.9MB.  RMS scale and RoPE
    applied on device; the per-column g vectors, RoPE signs, and the
    q-only 1/sqrt(d) attention scale are baked into per-position cos/sin
    tables on the host.  Output: normalized, roped q|v (or k|v) in bf16.
  Host glue: gathers rows, applies the (numpy, index-only) KV-cache
    roll/update/window logic of the reference, and builds the effective
    4680-key K^T / V tensors per head (old cache rows come straight from
    the cache input; softmax is permutation-invariant so key order is
    free). Everything is laid out/cast for launch 2.
  Launch 2 (2D head x query sharded): core (hg, qh) takes 3 heads x 780
    queries x all 4680 keys (per-core KV DMA 7.2MB instead of 28.7MB).
    Query blocks of 512/268 (one PSUM bank each), logits^T = K^T-chunk . Q
    (keys on partitions), one exp per 3-chunk group on ScalarE, P.V
    accumulated per block, opsum staged out of PSUM immediately (bank
    frees before the denominator chain), softmax denominators via a bf16
    pairwise tree over the resident P tile (VectorE 2x) + gpsimd
    partition_all_reduce + reciprocal, then a per-head-group PARTIAL
    out-projection o3 @ wo_rows; the host sums the 4 head-group partials
    per query half and adds the bias (free).

  Measured on trn2 (8 cores): launch1 ~88us + launch2 ~159us =
  ~248us total HW exec; relative error vs the fp32 reference ~5.5e-3
  (all matmul operands bf16, fp32 accumulation).
"""

import os
import sys

for _p in ("/opt/trn_rl_repo",):
    if os.path.isdir(_p) and _p not in sys.path:
        sys.path.insert(0, _p)

import numpy as np
import ml_dtypes

import concourse.bass as bass
import concourse.tile as tile
from concourse import bacc
from concourse import mybir
from concourse import bass_utils
from concourse import bass_isa
from concourse.alu_op_type import AluOpType

BF16 = ml_dtypes.bfloat16
AF = mybir.ActivationFunctionType

# ---------------------------------------------------------------------------
# Problem constants (fixed by the input specs).
S = 1560          # query/new-token sequence length
DIM = 1536
NH = 12
HD = 128
CACHE = 4680      # kv cache length == effective attention keys here
NCORES = 8
RPC = S // NCORES  # 195 rows (queries) per core
EPS = 1e-6
LOCAL_ATTN_SIZE = 3
SINK_SIZE = 1
MAX_ATTN = 32760 if LOCAL_ATTN_SIZE == -1 else LOCAL_ATTN_SIZE * S

NKC = (CACHE + 127) // 128      # 37 key chunks
TAIL = CACHE - (NKC - 1) * 128  # 72 keys in the tail chunk

# Row-chunk split of the 195 per-core rows into <=128-partition chunks.
RCHUNKS = [(0, 128), (128, 195)]

_CACHED = {}
LAST_RUNS = []  # BassKernelResults of the most recent kernel() call


# ---------------------------------------------------------------------------
# Launch 1 (pair-split): cores 2p/2p+1 share query rows [390p, 390p+390).
# Core 2p computes all of wq plus wv[:, :768]; core 2p+1 all of wk plus
# wv[:, 768:].  Each core's normed tensor (q or k) is column-complete, so
# the RMS norm stays core-local, and per-core weight DMA drops from
# 14.2MB (replicated W) to ~7.9MB.  The program is SPMD: the q-only
# 1/sqrt(HD) attention scale is folded into the host-built Q rope tables.
RPB = 390                 # rows per core (pair rows)
RCH4 = [(0, 128), (128, 256), (256, 384), (384, 390)]
NTW = [512, 512, 512, 512, 256]   # 3 normed subtiles + 1.5 plain subtiles
CPC = 2304                # output columns per core


def _build_launch1():
    nc = bacc.Bacc("TRN2", target_bir_lowering=False, debug=False,
                   num_devices=NCORES, num_swdge_queues=4)
    f32, bf = mybir.dt.float32, mybir.dt.bfloat16

    xt_d = nc.dram_tensor("xt", [128, 12, RPB], bf, kind="ExternalInput")
    w3_d = nc.dram_tensor("w3", [5, 128, 12, 512], bf, kind="ExternalInput")
    ct_d = nc.dram_tensor("ct", [RPB, DIM], bf, kind="ExternalInput")
    st_d = nc.dram_tensor("st", [RPB, DIM], bf, kind="ExternalInput")
    out_d = nc.dram_tensor("qkv", [RPB, CPC], bf, kind="ExternalOutput")

    with tile.TileContext(nc) as tc:
        with (
            tc.tile_pool(name="consts", bufs=1) as consts,
            tc.tile_pool(name="wstream", bufs=3) as wstream,
            tc.tile_pool(name="stage", bufs=1) as stagep,
            tc.tile_pool(name="ps", bufs=4, space="PSUM") as psp,
            tc.tile_pool(name="small", bufs=2) as small,
            tc.tile_pool(name="outs", bufs=1) as outsp,
            tc.tile_pool(name="tmp", bufs=1) as tmpp,
        ):
            xt = consts.tile([128, 12, RPB], bf)
            # split so the first k-chunks land early; W tile 0 streams on
            # scalar concurrently (the 22us serialized startup was the
            # largest single stall in the launch-1 trace)
            for kc3 in range(3):
                nc.sync.dma_start(xt[:, 4 * kc3:4 * kc3 + 4, :],
                                  xt_d.ap()[:, 4 * kc3:4 * kc3 + 4, :])

            # fp32 staging for the normed tensor's rows (post-matmul)
            stage = {ri: stagep.tile([r1 - r0, DIM], f32, tag=f"st{ri}",
                                     name=f"st{ri}")
                     for ri, (r0, r1) in enumerate(RCH4)}

            # per (rchunk, nsub) partial sums of squares
            ssq = {}
            for ri, (r0, r1) in enumerate(RCH4):
                for ns in range(3):
                    ssq[(ri, ns)] = small.tile(
                        [r1 - r0, 1], f32, tag=f"ssq{ri}{ns}",
                        name=f"ssq{ri}{ns}")

            outt = {ri: outsp.tile([r1 - r0, CPC], bf, tag=f"out{ri}",
                                   name=f"out{ri}")
                    for ri, (r0, r1) in enumerate(RCH4)}

            sq_scratch = {ri: tmpp.tile([r1 - r0, 512], bf, tag=f"sqs{ri}",
                                        name=f"sqs{ri}")
                          for ri, (r0, r1) in enumerate(RCH4)}

            epsb = consts.tile([128, 1], f32, name="epsb")
            nc.vector.memset(epsb[:], EPS)

            tabs = {}
            tab_specs = [(name, dram, ri)
                         for name, dram in (("ct", ct_d), ("st", st_d))
                         for ri in range(len(RCH4))]

            # normalization scale + rope + cast for one row-chunk of the
            # normed (T) section; called once its three subtiles are done
            def finish_T(ri):
                r0, r1 = RCH4[ri]
                rs = r1 - r0
                st = stage[ri]
                tot = small.tile([rs, 1], f32, tag=f"tot{ri}",
                                 name=f"tot{ri}")
                nc.vector.tensor_tensor(
                    tot[:], ssq[(ri, 0)][:rs, :], ssq[(ri, 1)][:rs, :],
                    AluOpType.add)
                nc.vector.tensor_tensor(
                    tot[:], tot[:], ssq[(ri, 2)][:rs, :], AluOpType.add)
                nc.scalar.activation(out=tot[:], in_=tot[:], func=AF.Sqrt,
                                     bias=epsb[:rs, :], scale=1.0 / DIM)
                nc.vector.reciprocal(out=tot[:], in_=tot[:])

                # rope: y = x*C + swap(x)*S'  (signs + any attention scale
                # are folded into the host-built tables)
                sw = tmpp.tile([rs, DIM], f32, tag=f"sw{ri}", name=f"sw{ri}")
                st3 = st[:rs, :].rearrange("p (c two) -> p c two", two=2)
                sw3 = sw[:rs, :].rearrange("p (c two) -> p c two", two=2)
                nc.scalar.copy(sw3[:, :, 0], st3[:, :, 1])
                nc.scalar.copy(sw3[:, :, 1], st3[:, :, 0])
                t1 = tmpp.tile([rs, DIM], f32, tag=f"t1{ri}", name=f"t1{ri}")
                nc.vector.tensor_tensor(
                    t1[:], st[:rs, :], tabs[("ct", ri)][:], AluOpType.mult)
                nc.vector.tensor_tensor(
                    sw[:rs, :], sw[:rs, :], tabs[("st", ri)][:],
                    AluOpType.mult)
                nc.vector.tensor_tensor(
                    t1[:], t1[:], sw[:rs, :], AluOpType.add)
                # scale by rms (per partition) and cast to bf16
                nc.scalar.activation(
                    out=outt[ri][:rs, 0:DIM],
                    in_=t1[:], func=AF.Copy, scale=tot[:])
                # store the T section immediately (sync/scalar queues are
                # idle by now); the V section goes out in the final block
                r0b, r1b = RCH4[ri]
                nc.sync.dma_start(out_d.ap()[r0b:r1b, 0:768],
                                  outt[ri][:, 0:768])
                nc.scalar.dma_start(out_d.ap()[r0b:r1b, 768:DIM],
                                    outt[ri][:, 768:DIM])

            for n in range(5):
                wn = NTW[n]
                wt = wstream.tile([128, 12, 512], bf, tag="w", name="wt")
                eng = (nc.scalar, nc.gpsimd, nc.sync)[n % 3]
                eng.dma_start(wt[:], w3_d.ap()[n])
                # slip two rope-table loads in behind each W tile
                if n >= 1:
                    for sl in range(2):
                        if not tab_specs:
                            break
                        name, dram, ri = tab_specs.pop(0)
                        r0, r1 = RCH4[ri]
                        t = consts.tile([r1 - r0, DIM], bf,
                                        tag=f"tab{name}{ri}",
                                        name=f"tab{name}{ri}")
                        (nc.scalar if (n + sl) % 2 else nc.gpsimd).dma_start(
                            t[:], dram.ap()[r0:r1, :])
                        tabs[(name, ri)] = t
                for ri, (r0, r1) in enumerate(RCH4):
                    rs = r1 - r0
                    pr = psp.tile([128, 512], f32, tag="pr", name="pr")
                    for kc in range(12):
                        nc.tensor.matmul(
                            pr[:rs, :wn],
                            xt[:, kc, r0:r1],
                            wt[:, kc, :wn],
                            start=(kc == 0),
                            stop=(kc == 11),
                        )
                    if n < 3:
                        # partial sum of squares for RMS (ScalarE)
                        nc.scalar.activation(
                            out=sq_scratch[ri][:rs, :wn],
                            in_=pr[:rs, :wn],
                            func=AF.Square,
                            accum_out=ssq[(ri, n)][:rs, :],
                        )
                        # stage fp32 for rope (VectorE copy)
                        nc.vector.tensor_copy(
                            stage[ri][:rs, n * 512:(n + 1) * 512],
                            pr[:rs, :wn],
                        )
                    else:
                        # plain (v) columns: cast straight to the output
                        nc.vector.tensor_copy(
                            outt[ri][:rs, DIM + (n - 3) * 512:
                                     DIM + (n - 3) * 512 + wn],
                            pr[:rs, :wn],
                        )
                        # last W tile: this row-chunk's T section is final,
                        # so its normalize+rope overlaps the remaining
                        # row-chunks' matmuls (all W/table DMA triggers are
                        # already queued, so none sits behind these ops)
                        if n == 4:
                            finish_T(ri)

            # V-section stores (T sections go out inside finish_T)
            for ri, (r0, r1) in enumerate(RCH4):
                nc.gpsimd.dma_start(
                    out_d.ap()[r0:r1, DIM:CPC],
                    outt[ri][:, DIM:CPC])

    nc.finalize()
    return nc


# ---------------------------------------------------------------------------
# Launch 2 (2D-sharded): core (hg, qh) handles 3 heads x 780 queries x all
# 4680 keys.  Per-core KV DMA drops 28.7MB -> 7.2MB, QK/PV moving width
# rises to 512/268, denominators via one VectorE tree-reduce over the
# resident P tile + gpsimd partition_all_reduce (no PSUM/ones-matmuls),
# and the out-projection is computed as a per-head-group partial that the
# host sums for free.
HPC = 3            # heads per core
QPC = 780          # queries per core
QB = (512, 268)    # query blocks (one PSUM bank each)
NGR = 13           # 13 exp groups of 3 key chunks (last group = 1 chunk)


def _build_launch2():
    nc = bacc.Bacc("TRN2", target_bir_lowering=False, debug=False,
                   num_devices=NCORES, num_swdge_queues=4)
    f32, bf = mybir.dt.float32, mybir.dt.bfloat16

    qt_d = nc.dram_tensor("qt", [128, HPC, 784], bf, kind="ExternalInput")
    kt_d = nc.dram_tensor("kt", [HPC, 128, CACHE], bf, kind="ExternalInput")
    vt_d = nc.dram_tensor("vt", [HPC, 128, NKC, 128], bf, kind="ExternalInput")
    w2_d = nc.dram_tensor("w2", [128, HPC, 3, 512], bf, kind="ExternalInput")
    out_d = nc.dram_tensor("outp", [QPC, DIM], bf, kind="ExternalOutput")

    with tile.TileContext(nc) as tc:
        with (
            tc.tile_pool(name="consts", bufs=1) as consts,
            tc.tile_pool(name="kv", bufs=3) as kvp,
            tc.tile_pool(name="p", bufs=2) as pp,
            tc.tile_pool(name="acc", bufs=1) as accp,
            tc.tile_pool(name="lp", bufs=2, space="PSUM") as lpp,
            tc.tile_pool(name="ops", bufs=2, space="PSUM") as opsp,
            tc.tile_pool(name="outs", bufs=2) as outsp,
            tc.tile_pool(name="o3u", bufs=2) as o3up,
        ):
            qt = consts.tile([128, HPC, 784], bf)
            # per-head slices so the first QK only waits on head 0's queries
            for qh_ in range(HPC):
                nc.scalar.dma_start(qt[:, qh_, :], qt_d.ap()[:, qh_, :])
            w2 = consts.tile([128, HPC, 3, 512], bf)
            nc.scalar.dma_start(w2[:], w2_d.ap())
            o3 = consts.tile([128, HPC, 784], bf)  # normalized o^T per head
            # PE warmup: trip the HAM clock gate to 8/8 before the stream
            wsrc = consts.tile([128, 512], bf, name="wsrc")
            nc.vector.memset(wsrc[:], 0.0)
            for wu in range(24):
                wp = lpp.tile([128, 3, 512], f32, tag="lp", name="lpw")
                nc.tensor.matmul(wp[:, 0, :], wsrc[:, :128], wsrc[:],
                                 start=True, stop=True)

            # den tree scratch (bf16 pairwise tree over the resident P tile)
            acc1 = accp.tile([128, 18, 512], bf, name="acc1")
            acc2 = accp.tile([128, 9, 512], bf, name="acc2")
            acc3 = accp.tile([128, 4, 512], bf, name="acc3")
            acc4 = accp.tile([128, 2, 512], bf, name="acc4")
            acc5 = accp.tile([128, 512], bf, name="acc5")
            acc6 = accp.tile([128, 512], bf, name="acc6")
            den = accp.tile([128, 512], f32, name="den")
            sden = accp.tile([128, 512], f32, name="sden")
            sinv = accp.tile([128, 512], f32, name="sinv")

            pending_pv = []   # software-pipeline lag: QK group g+1 before PV g

            def flush_pv():
                while pending_pv:
                    pending_pv.pop(0)()

            # all K/V loads upfront (resident for both query blocks); the
            # triggers sit ahead of every partition_all_reduce in the
            # gpsimd FIFO so DMAs never wait on compute
            kts, vts = {}, {}
            for h in range(HPC):
                kts[h] = kvp.tile([128, CACHE], bf, tag="kt",
                                  name=f"ktile{h}")
                vts[h] = kvp.tile([128, NKC, 128], bf, tag="vt",
                                  name=f"vtile{h}")
            nc.sync.dma_start(kts[0][:, :512], kt_d.ap()[0][:, :512])
            nc.gpsimd.dma_start(vts[0][:, :6, :], vt_d.ap()[0][:, :6, :])
            nc.sync.dma_start(kts[0][:, 512:1536], kt_d.ap()[0][:, 512:1536])
            nc.sync.dma_start(kts[0][:, 1536:], kt_d.ap()[0][:, 1536:])
            nc.gpsimd.dma_start(vts[0][:, 6:, :], vt_d.ap()[0][:, 6:, :])
            for h in range(1, HPC):
                nc.sync.dma_start(kts[h][:], kt_d.ap()[h])
                nc.gpsimd.dma_start(vts[h][:], vt_d.ap()[h])

            def out_proj(qchunks):
                for qi, (r0, r1) in qchunks:
                    rs = r1 - r0
                    for nf in range(3):
                        po = lpp.tile([128, 3, 512], f32, tag="lp",
                                      name="po")
                        for h in range(HPC):
                            nc.tensor.matmul(
                                po[:rs, 0, :],
                                o3[:, h, r0:r1],
                                w2[:, h, nf, :],
                                start=(h == 0), stop=(h == HPC - 1))
                        outf = outsp.tile([128, 512], bf, tag="of",
                                          name="of")
                        # ScalarE is idle at out-proj time; VectorE still
                        # has the last denominator tree queued ahead
                        nc.scalar.copy(outf[:rs, :], po[:rs, 0, :])
                        deng = (nc.sync, nc.scalar, nc.gpsimd)[nf]
                        deng.dma_start(
                            out_d.ap()[r0:r1, nf * 512:(nf + 1) * 512],
                            outf[:rs, :])

            for qb in range(2):
                qw = QB[qb]
                q0 = 512 * qb
                for h in range(HPC):
                    kt, vt = kts[h], vts[h]
                    opsum = opsp.tile([128, 512], f32, tag="opsum",
                                      name="opsum")
                    pt = pp.tile([128, NKC, 512], bf, tag="pt", name="pt")
                    for g in range(NGR):
                        jj = 3 * g
                        nch = min(3, NKC - jj)
                        lp = lpp.tile([128, 3, 512], f32, tag="lp", name="lp")
                        for u in range(nch):
                            j = jj + u
                            kw = TAIL if j == NKC - 1 else 128
                            if kw < 128:
                                # full-slot memset (partition base must be
                                # aligned); the matmul overwrites rows 0:kw
                                nc.vector.memset(lp[:, u, :qw], -1e30)
                            nc.tensor.matmul(
                                lp[:kw, u, :qw],
                                kt[:, j * 128:j * 128 + kw],
                                qt[:, h, q0:q0 + qw],
                                start=True, stop=True)
                        nc.scalar.activation(
                            out=pt[:, jj:jj + nch, :qw],
                            in_=lp[:, :nch, :qw], func=AF.Exp)

                        def pv_group(jj=jj, nch=nch, pt=pt, vt=vt,
                                     opsum=opsum, qw=qw):
                            for u in range(nch):
                                j = jj + u
                                nc.tensor.matmul(
                                    opsum[:, :qw],
                                    vt[:, j, :],
                                    pt[:, j, :qw],
                                    start=(j == 0), stop=(j == NKC - 1))
                        pending_pv.append(pv_group)
                        if len(pending_pv) > 1:
                            pending_pv.pop(0)()
                    flush_pv()

                    # stage opsum out of PSUM right away so the bank frees
                    # before the (long) denominator chain completes
                    o3u = o3up.tile([128, 512], f32, tag="o3u", name="o3u")
                    nc.vector.tensor_copy(o3u[:, :qw], opsum[:, :qw])

                    # denominator: bf16 pairwise tree over pt (VectorE, 2x)
                    TT = nc.vector.tensor_tensor
                    TT(acc1[:, :, :qw], pt[:, 0:18, :qw], pt[:, 18:36, :qw],
                       AluOpType.add)
                    TT(acc2[:, :, :qw], acc1[:, 0:9, :qw], acc1[:, 9:18, :qw],
                       AluOpType.add)
                    TT(acc3[:, :, :qw], acc2[:, 0:4, :qw], acc2[:, 4:8, :qw],
                       AluOpType.add)
                    TT(acc4[:, :, :qw], acc3[:, 0:2, :qw], acc3[:, 2:4, :qw],
                       AluOpType.add)
                    TT(acc5[:, :qw], acc4[:, 0, :qw], acc4[:, 1, :qw],
                       AluOpType.add)
                    TT(acc6[:, :qw], acc5[:, :qw], acc2[:, 8, :qw],
                       AluOpType.add)
                    TT(den[:, :qw], acc6[:, :qw], pt[:, 36, :qw],
                       AluOpType.add)
                    nc.gpsimd.partition_all_reduce(
                        sden[:, :qw], den[:, :qw], 128,
                        bass_isa.ReduceOp.add)
                    nc.vector.reciprocal(out=sinv[:, :qw], in_=sden[:, :qw])
                    nc.vector.tensor_tensor(
                        o3[:, h, q0:q0 + qw], o3u[:, :qw], sinv[:, :qw],
                        AluOpType.mult)

            out_proj(list(enumerate(
                [(0, 128), (128, 256), (256, 384), (384, 512),
                 (512, 640), (640, 768), (768, 780)])))

    nc.finalize()
    return nc


# ---------------------------------------------------------------------------
def _cache_plan(current_start, global_end_index, local_end_index, s, kv_size,
                frame_seqlen):
    """Numpy re-implementation of the reference's cache roll/update/window
    logic, tracking only *indices*: returns (old_cache_rows, new_rows) such
    that the attended key set == cache[old_cache_rows] ++ new[new_rows]."""
    current_end = current_start + s
    sink_tokens = SINK_SIZE * frame_seqlen

    # each cache slot: kind 0 -> original cache row idx, kind 1 -> new row idx
    kind = np.zeros(kv_size, dtype=np.int64)
    idx = np.arange(kv_size, dtype=np.int64)

    if (LOCAL_ATTN_SIZE != -1 and current_end > global_end_index
            and s + local_end_index > kv_size):
        num_evicted = s + local_end_index - kv_size
        num_rolled = local_end_index - num_evicted - sink_tokens
        src0 = sink_tokens + num_evicted
        kind[sink_tokens:sink_tokens + num_rolled] = \
            kind[src0:src0 + num_rolled]
        idx[sink_tokens:sink_tokens + num_rolled] = \
            idx[src0:src0 + num_rolled]
        new_local_end = (local_end_index + current_end - global_end_index
                         - num_evicted)
    else:
        new_local_end = local_end_index + current_end - global_end_index
    local_start = new_local_end - s
    is_recompute = (current_end <= global_end_index) and (current_start > 0)
    write_start = max(local_start, sink_tokens) if is_recompute \
        else local_start
    off = max(0, write_start - local_start)
    wl = max(0, new_local_end - write_start)
    if wl > 0:
        kind[write_start:new_local_end] = 1
        idx[write_start:new_local_end] = off + np.arange(wl)

    if sink_tokens > 0:
        budget = MAX_ATTN - sink_tokens
        if budget > 0:
            lo = max(sink_tokens, new_local_end - budget)
            sel = np.concatenate([np.arange(sink_tokens),
                                  np.arange(lo, new_local_end)])
        else:
            sel = np.arange(sink_tokens)
    else:
        ws = max(0, new_local_end - MAX_ATTN)
        sel = np.arange(ws, new_local_end)

    k_kind, k_idx = kind[sel], idx[sel]
    old_rows = k_idx[k_kind == 0]
    new_rows = k_idx[k_kind == 1]
    return old_rows, new_rows


def _rope_tables(freqs_real, freqs_imag, f, h, w, start_frame, gq, gk):
    """(S,DIM) cos table and sign-folded sin tables with g baked in."""
    c = HD // 2  # 64
    c0 = c - 2 * (c // 3)
    c1 = c // 3
    fr = np.asarray(freqs_real, np.float32)
    fi = np.asarray(freqs_imag, np.float32)
    s = f * h * w
    assert s == S
    fidx = np.arange(s) // (h * w)
    hidx = (np.arange(s) // w) % h
    widx = np.arange(s) % w
    fr_pos = np.concatenate([
        fr[start_frame + fidx][:, :c0],
        fr[hidx][:, c0:c0 + c1],
        fr[widx][:, c0 + c1:c0 + 2 * c1],
    ], axis=1)  # (S, 64)
    fi_pos = np.concatenate([
        fi[start_frame + fidx][:, :c0],
        fi[hidx][:, c0:c0 + c1],
        fi[widx][:, c0 + c1:c0 + 2 * c1],
    ], axis=1)
    # expand to per-column tables over one head, then tile across heads
    C1 = np.repeat(fr_pos, 2, axis=1)              # (S, 128)
    Sg = np.empty((s, HD), np.float32)
    Sg[:, 0::2] = -fi_pos                          # y_even = xe*c - xo*si
    Sg[:, 1::2] = fi_pos                           # y_odd  = xo*c + xe*si
    C = np.tile(C1, (1, NH))                       # (S, DIM)
    Sx = np.tile(Sg, (1, NH))
    gq = np.asarray(gq, np.float32)
    gk = np.asarray(gk, np.float32)
    gq_sw = gq.reshape(-1, 2)[:, ::-1].reshape(-1)
    gk_sw = gk.reshape(-1, 2)[:, ::-1].reshape(-1)
    return (C * gq[None, :], Sx * gq_sw[None, :],
            C * gk[None, :], Sx * gk_sw[None, :])


# ---------------------------------------------------------------------------
def kernel(x, cache_k, cache_v, freqs_real, freqs_imag,
           wq, bq, wk, bk, wv, bv, wo, bo, gq, gk,
           f_frames, height, width, current_start, global_end_index,
           local_end_index):
    global LAST_RUNS
    LAST_RUNS = []

    x = np.asarray(x, np.float32)
    cache_k = np.asarray(cache_k, np.float32)
    cache_v = np.asarray(cache_v, np.float32)
    wq = np.asarray(wq, np.float32)
    wk = np.asarray(wk, np.float32)
    wv = np.asarray(wv, np.float32)
    wo = np.asarray(wo, np.float32)
    bo = np.asarray(bo, np.float32)
    f = int(f_frames)
    h = int(height)
    w = int(width)
    current_start = int(current_start)
    global_end_index = int(global_end_index)
    local_end_index = int(local_end_index)

    assert x.shape == (1, S, DIM)
    for b in (bq, bk, bv):
        assert not np.any(np.asarray(b)), "nonzero qkv bias unsupported"

    frame_seqlen = h * w
    start_frame = current_start // frame_seqlen

    # ---- launch 1: projections + RMS + RoPE (pair-split) ----
    Cq, Sq, Ck, Sk = _rope_tables(freqs_real, freqs_imag, f, h, w,
                                  start_frame, gq, gk)
    att_sc = 1.0 / float(np.sqrt(HD))   # q-only scale, folded into tables

    def _w5(W):
        Wp = np.zeros((DIM, 2560), np.float32)
        Wp[:, :CPC] = W
        return np.ascontiguousarray(
            Wp.reshape(12, 128, 5, 512).transpose(2, 1, 0, 3)).astype(BF16)

    w5A = _w5(np.concatenate([wq, wv[:, :768]], axis=1))
    w5B = _w5(np.concatenate([wk, wv[:, 768:]], axis=1))
    xT = x[0].T.astype(BF16)                                # (1536, 1560)

    nc1 = _CACHED.get("l1")
    if nc1 is None:
        nc1 = _CACHED["l1"] = _build_launch1()

    in_maps1 = []
    for c in range(NCORES):
        p = c // 2
        r0, r1 = p * RPB, (p + 1) * RPB
        xt_c = np.ascontiguousarray(
            xT[:, r0:r1].reshape(12, 128, RPB).transpose(1, 0, 2))
        if c % 2 == 0:
            in_maps1.append({
                "xt": xt_c, "w3": w5A,
                "ct": np.ascontiguousarray(Cq[r0:r1] * att_sc).astype(BF16),
                "st": np.ascontiguousarray(Sq[r0:r1] * att_sc).astype(BF16),
            })
        else:
            in_maps1.append({
                "xt": xt_c, "w3": w5B,
                "ct": np.ascontiguousarray(Ck[r0:r1]).astype(BF16),
                "st": np.ascontiguousarray(Sk[r0:r1]).astype(BF16),
            })
    res1 = bass_utils.run_bass_kernel_spmd(nc1, in_maps1,
                                           core_ids=list(range(NCORES)))
    LAST_RUNS.append(res1)
    Q = np.concatenate(
        [res1.results[2 * p]["qkv"][:, :DIM] for p in range(4)], axis=0)
    Knew = np.concatenate(
        [res1.results[2 * p + 1]["qkv"][:, :DIM] for p in range(4)], axis=0)
    Vnew = np.concatenate(
        [np.concatenate([res1.results[2 * p]["qkv"][:, DIM:],
                         res1.results[2 * p + 1]["qkv"][:, DIM:]], axis=1)
         for p in range(4)], axis=0)

    # ---- host glue: effective K/V assembly ----
    old_rows, new_rows = _cache_plan(current_start, global_end_index,
                                     local_end_index, S, cache_k.shape[1],
                                     frame_seqlen)
    n_keys = len(old_rows) + len(new_rows)
    assert n_keys == CACHE, f"unexpected key count {n_keys}"

    K_eff = np.concatenate([
        cache_k[0, old_rows].reshape(len(old_rows), DIM).astype(BF16),
        Knew[new_rows],
    ], axis=0)  # (4680, 1536) bf16  (head-major columns)
    V_eff = np.concatenate([
        cache_v[0, old_rows].reshape(len(old_rows), DIM).astype(BF16),
        Vnew[new_rows],
    ], axis=0)

    kt = np.ascontiguousarray(K_eff.T.reshape(NH, HD, CACHE))
    V_pad = np.zeros((NKC * 128, DIM), BF16)
    V_pad[:CACHE] = V_eff
    vt = np.ascontiguousarray(
        V_pad.reshape(NKC, 128, NH, HD).transpose(2, 1, 0, 3))
    w2 = np.ascontiguousarray(
        wo.reshape(12, 128, 3, 512).transpose(1, 0, 2, 3)).astype(BF16)

    nc2 = _CACHED.get("l2")
    if nc2 is None:
        nc2 = _CACHED["l2"] = _build_launch2()

    in_maps2 = []
    for c in range(NCORES):
        hg, qh = c // 2, c % 2
        h0 = hg * HPC
        r0, r1 = qh * QPC, (qh + 1) * QPC
        qt_c = np.zeros((128, HPC, 784), BF16)
        qt_c[:, :, :QPC] = (
            Q[r0:r1, h0 * HD:(h0 + HPC) * HD].T
            .reshape(HPC, HD, QPC).transpose(1, 0, 2))
        in_maps2.append({
            "qt": qt_c,
            "kt": np.ascontiguousarray(kt[h0:h0 + HPC]),
            "vt": np.ascontiguousarray(vt[h0:h0 + HPC]),
            "w2": np.ascontiguousarray(w2[:, h0:h0 + HPC]),
        })
    res2 = bass_utils.run_bass_kernel_spmd(nc2, in_maps2,
                                           core_ids=list(range(NCORES)))
    LAST_RUNS.append(res2)

    out = np.zeros((S, DIM), np.float32)
    for c in range(NCORES):
        hg, qh = c // 2, c % 2
        out[qh * QPC:(qh + 1) * QPC] += \
            res2.results[c]["outp"].astype(np.float32)
    out += bo.reshape(1, DIM)
    return out.reshape(1, S, DIM)

